# revision 1
# baseline (speedup 1.0000x reference)
"""PointHop octant-binning kernel for TRN2 (8 NeuronCores, B-sharded).

v4 design (all rates HW-measured):
- masks via tensor_scalar is_gt (bf16 4x DVE mode)
- masked products via tensor_tensor mult (bf16 2x), broadcast-merged
- half the lattice via relu: relu(m*x) = m*relu(x) (tensor_scalar_max, 4x)
- k-reduction via binary halving tree: fp16 TT adds (2x) for the first 3
  levels, f32 for the last 3 (precision), one instruction per level for
  all arrays; counts subtree runs on GPSIMD in parallel
- epilogue (Moebius butterfly, counts, means, std) batched over 4 slabs

MEGA m-layout: m0-23 value lattice (q-major, u=4bx+2by+bz), m24-26
squares, m27-33 count masks u1..u7.
"""

import os
from contextlib import ExitStack

import numpy as np

if "axon" not in os.environ.get("JAX_PLATFORMS", "axon"):
    os.environ.pop("JAX_PLATFORMS", None)

import concourse.bass as bass
import concourse.bacc as bacc
import concourse.tile as tile
from concourse import mybir
from concourse.bass_utils import run_bass_kernel_spmd

B, C, N, K = 32, 3, 8192, 64
NCORES = 8
BL = B // NCORES          # 4 batches per core
PART = 128
TG = 8                    # groups per partition per slab
SLAB = PART * TG          # 1024 groups per slab
NSLAB = BL * N // SLAB    # 32 slabs per core
SB = 8                    # slabs per epilogue batch
NBATCH = NSLAB // SB
FOUT = 30
SEC = TG * K              # 512 elems per section
V32 = SB * TG             # batched (s, t) dim
NMV = 27                  # value+square arrays (DVE tree)
NMC = 7                   # count arrays (GPSIMD tree)
NM = NMV + NMC

AL = mybir.AluOpType
AF = mybir.ActivationFunctionType
F32 = mybir.dt.float32
FP16 = mybir.dt.float16


def _build_kernel(nc: bass.Bass):
    gx = nc.dram_tensor("gx", [BL, C, N, K], F32, kind="ExternalInput")
    nx = nc.dram_tensor("nx", [BL, N, C], F32, kind="ExternalInput")
    out = nc.dram_tensor("out", [BL, N, FOUT], F32, kind="ExternalOutput")

    with tile.TileContext(nc) as tc, ExitStack() as ctx:
        vpool = ctx.enter_context(tc.tile_pool(name="v", bufs=3))
        mpool = ctx.enter_context(tc.tile_pool(name="m", bufs=2))
        epool = ctx.enter_context(tc.tile_pool(name="e", bufs=2))

        ts = nc.vector.tensor_scalar
        tt = nc.vector.tensor_tensor
        act = nc.scalar.activation

        for batch in range(NBATCH):
            EP = epool.tile([PART, NM * V32], F32, name="EP")   # (m, s, t)
            CT = epool.tile([PART, 8 * V32], F32, name="CT")    # (u, s, t)
            CIN = epool.tile([PART, SB * TG * C], F32, name="CIN")
            O = epool.tile([PART, SB * TG * FOUT], F32, name="O")
            nc.gpsimd.memset(CT[:, 0:V32], float(K))  # count u0 = K

            ep4 = EP[:].rearrange("p (m s t) -> p m s t", m=NM, s=SB)
            ct4 = CT[:].rearrange("p (u s t) -> p u s t", u=8, s=SB)

            for sl in range(SB):
                slab = batch * SB + sl
                b, s = divmod(slab, N // SLAB)
                n0 = s * SLAB

                V = vpool.tile([PART, C * SEC], F32, name="V")
                nc.sync.dma_start(
                    out=V[:].rearrange("p (c t k) -> p c t k", c=C, t=TG),
                    in_=gx[b, :, n0:n0 + SLAB, :].rearrange(
                        "c (p t) k -> p c t k", p=PART, t=TG))
                nc.sync.dma_start(
                    out=CIN[:, sl * TG * C:(sl + 1) * TG * C].rearrange(
                        "p (t c) -> p t c", t=TG),
                    in_=nx[b, n0:n0 + SLAB, :].rearrange(
                        "(p t) c -> p t c", p=PART, t=TG))

                MEGA = mpool.tile([PART, NM * SEC], FP16, name="MEGA")

                def M(m, nsec=1):
                    return MEGA[:, m * SEC:(m + nsec) * SEC]

                vq = V[:].rearrange("p (c s) -> p c s", c=C)
                mq = MEGA[:, 0:24 * SEC].rearrange(
                    "p (q u s) -> p q u s", q=3, u=8)
                act(mq[:, :, 0, :], vq, AF.Copy)      # cast x,y,z -> u0
                act(M(24, 3), V[:], AF.Square)        # squares m24-26

                # count masks (4x): [m27,m28]=[mz,my] via one
                # negative-stride op; m30=mx
                ts(MEGA[:, 27 * SEC:29 * SEC].rearrange(
                       "p (a s) -> p a s", a=2),
                   MEGA[:, 0:24 * SEC].rearrange(
                       "p (a b s) -> p a b s", a=3, b=8)[:, 2:0:-1, 0, :],
                   0.0, None, AL.is_gt)
                ts(M(30), M(0), 0.0, None, AL.is_gt)
                def bcast(ap, n):
                    return ap[:, None, :].broadcast_to([PART, n, SEC])

                # composites: m29=my*mz, then [m31,m32,m33] =
                # [mz,my,myz] * mx (one broadcast TT)
                tt(M(29), M(27), M(28), AL.mult)
                tt(M(31, 3).rearrange("p (a s) -> p a s", a=3),
                   MEGA[:, 27 * SEC:30 * SEC].rearrange(
                       "p (a s) -> p a s", a=3),
                   bcast(M(30), 3), AL.mult)

                # value products via TT (2x), u = 4bx+2by+bz
                # s1: mz * [x, y] -> q0u1(m1), q1u1(m9)
                d1 = MEGA[:, SEC:17 * SEC].rearrange(
                    "p (a b s) -> p a b s", a=2, b=8)[:, :, 0, :]
                s1 = MEGA[:, 0:16 * SEC].rearrange(
                    "p (a b s) -> p a b s", a=2, b=8)[:, :, 0, :]
                tt(d1, s1, bcast(M(27), 2), AL.mult)
                # s2: my * [x, mz*x] -> m2, m3
                tt(M(2, 2).rearrange("p (a s) -> p a s", a=2),
                   M(0, 2).rearrange("p (a s) -> p a s", a=2),
                   bcast(M(28), 2), AL.mult)
                # s3: my * z -> q2u2 (m18)
                tt(M(18), M(16), M(28), AL.mult)
                # s4: mx * [y, mz*y] -> q1u4, q1u5 (m12, m13)
                tt(M(12, 2).rearrange("p (a s) -> p a s", a=2),
                   M(8, 2).rearrange("p (a s) -> p a s", a=2),
                   bcast(M(30), 2), AL.mult)
                # s5: mx * [z, my*z] -> q2u4(m20), q2u6(m22)
                d5 = MEGA[:, 20 * SEC:24 * SEC].rearrange(
                    "p (a b s) -> p a b s", a=2, b=2)[:, :, 0, :]
                s5 = MEGA[:, 16 * SEC:20 * SEC].rearrange(
                    "p (a b s) -> p a b s", a=2, b=2)[:, :, 0, :]
                tt(d5, s5, bcast(M(30), 2), AL.mult)

                # relus: x-block u4-7 <- relu(u0-3) (ACT)
                act(M(4, 4), M(0, 4), AF.Relu)
                yb = MEGA[:, 8 * SEC:16 * SEC].rearrange(
                    "p (h d s) -> p h d s", h=2, d=2)
                act(yb[:, :, 1, :], yb[:, :, 0, :], AF.Relu)
                zb = MEGA[:, 16 * SEC:24 * SEC].rearrange(
                    "p (h d s) -> p h d s", h=4, d=2)
                act(zb[:, :, 1, :], zb[:, :, 0, :], AF.Relu)

                # ---- k-reduction tree (all 34 arrays, DVE) ----
                MT = NM * TG  # 272 (m,t) rows
                T1 = mpool.tile([PART, MT * 32], FP16, name="T1")
                T2 = mpool.tile([PART, MT * 4], FP16, name="T2")
                mv = MEGA[:].rearrange("p (m h k) -> p m h k", m=MT, h=2)
                t1v = T1[:].rearrange("p (m k) -> p m k", m=MT)
                tt(t1v, mv[:, :, 0, :], mv[:, :, 1, :], AL.add)       # L1
                t1h = T1[:].rearrange("p (m h k) -> p m h k", m=MT, h=2)
                tt(t1h[:, :, 0, :], t1h[:, :, 0, :], t1h[:, :, 1, :],
                   AL.add)                                            # L2
                t1q = T1[:].rearrange("p (m h k) -> p m h k", m=MT, h=4)
                tt(t1q[:, :, 0, :], t1q[:, :, 0, :], t1q[:, :, 1, :],
                   AL.add)                                            # L3
                t1o = T1[:].rearrange("p (m h k) -> p m h k", m=MT, h=8)
                t2v = T2[:].rearrange("p (m k) -> p m k", m=MT)
                tt(t2v, t1o[:, :, 0, :], t1o[:, :, 1, :], AL.add)     # L4
                t2h = T2[:].rearrange("p (m h k) -> p m h k", m=MT, h=2)
                tt(t2h[:, :, 0, :], t2h[:, :, 0, :], t2h[:, :, 1, :],
                   AL.add)                                            # L5
                t2q = T2[:].rearrange("p (m t k) -> p m t k", m=NM, t=TG)
                tt(ep4[:, :, sl, :], t2q[:, :, :, 0], t2q[:, :, :, 1],
                   AL.add)                                            # L6

            # ---- batched epilogue (sum-space) ----
            act(CT[:, V32:8 * V32], EP[:, NMV * V32:NM * V32], AF.Copy)
            stv = EP[:, 0:24 * V32].rearrange(
                "p (c u v) -> p c u v", c=3, u=8)
            Q = epool.tile([PART, 3 * V32], F32, name="Q")
            act(Q[:].rearrange("p (c v) -> p c v", c=3),
                stv[:, :, 0, :], AF.Square, 0.0, 1.0 / 8.0)

            sub = nc.vector.tensor_tensor
            sub(stv[:, :, 0:4, :], stv[:, :, 0:4, :], stv[:, :, 4:8, :],
                AL.subtract)
            st5 = EP[:, 0:24 * V32].rearrange(
                "p (m u v) -> p m u v", m=6, u=4)
            sub(st5[:, :, 0:2, :], st5[:, :, 0:2, :], st5[:, :, 2:4, :],
                AL.subtract)
            st6 = EP[:, 0:24 * V32].rearrange(
                "p (m u v) -> p m u v", m=12, u=2)
            sub(st6[:, :, 0:1, :], st6[:, :, 0:1, :], st6[:, :, 1:2, :],
                AL.subtract)

            ct3 = CT[:].rearrange("p (u v) -> p u v", u=8)
            sub(ct3[:, 0:4, :], ct3[:, 0:4, :], ct3[:, 4:8, :], AL.subtract)
            ctr4 = CT[:].rearrange("p (a u v) -> p a u v", a=2, u=4)
            sub(ctr4[:, :, 0:2, :], ctr4[:, :, 0:2, :], ctr4[:, :, 2:4, :],
                AL.subtract)
            ctr5 = CT[:].rearrange("p (a u v) -> p a u v", a=4, u=2)
            sub(ctr5[:, :, 0:1, :], ctr5[:, :, 0:1, :], ctr5[:, :, 1:2, :],
                AL.subtract)

            CC = epool.tile([PART, 8 * V32], F32, name="CC")
            RC = epool.tile([PART, 8 * V32], F32, name="RC")
            nc.vector.tensor_scalar_max(CC[:], CT[:], 1.0)
            nc.vector.reciprocal_approx_fast(RC[:], CC[:])

            ovb = O[:].rearrange("p (s t f) -> p s t f", s=SB, t=TG)
            mn5 = ovb[:, :, :, 6:30].rearrange(
                "p s t (u c) -> p s t u c", u=8)
            rcv = RC[:].rearrange("p (u s t) -> p s t u", u=8, s=SB)
            for c in range(3):
                stc = stv[:, c].rearrange("p u (s t) -> p s t u", s=SB)
                nc.gpsimd.tensor_tensor(mn5[:, :, :, :, c], stc, rcv,
                                        AL.mult)

            # std = sqrt((SS - Q)/63)
            D = epool.tile([PART, 3 * V32], F32, name="D")
            sub(D[:], EP[:, 24 * V32:27 * V32], Q[:], AL.subtract)
            act(ovb[:, :, :, 0:3],
                D[:].rearrange("p (c s t) -> p s t c", c=3, s=SB),
                AF.Sqrt, 0.0, 1.0 / 63.0)
            nc.gpsimd.tensor_copy(
                ovb[:, :, :, 3:6],
                CIN[:].rearrange("p (s t c) -> p s t c", s=SB, t=TG))

            bb, nb0 = batch, 0
            nc.sync.dma_start(
                out=out[bb, nb0:nb0 + SB * SLAB, :].rearrange(
                    "(s p t) f -> p s t f", s=SB, p=PART),
                in_=ovb)


_CACHE: dict = {}


def _get_nc():
    if "nc" not in _CACHE:
        nc = bacc.Bacc("TRN2", target_bir_lowering=False, debug=False)
        _build_kernel(nc)
        nc.finalize()
        _CACHE["nc"] = nc
    return _CACHE["nc"]


def kernel(group_xyz: np.ndarray, new_xyz: np.ndarray) -> np.ndarray:
    nc = _get_nc()
    gx = np.ascontiguousarray(group_xyz, dtype=np.float32)
    nx = np.ascontiguousarray(new_xyz, dtype=np.float32)
    in_maps = [
        {"gx": gx[i * BL:(i + 1) * BL], "nx": nx[i * BL:(i + 1) * BL]}
        for i in range(NCORES)
    ]
    res = run_bass_kernel_spmd(nc, in_maps, list(range(NCORES)))
    return np.concatenate([res.results[i]["out"] for i in range(NCORES)],
                          axis=0)



# revision 5
# speedup vs baseline: 1.4403x; 1.4403x over previous
"""PointHop octant-binning kernel for TRN2 (8 NeuronCores, B-sharded).

v5 design: [k,g]-transposed layout + PE (TensorEngine) reduction.

Per stripe of 1024 groups (32 stripes/core):
1. DMA gx -> V f32 [128 groups, (3c, 8t, 64k)]
2. ACT casts f32->fp16 (Vh)
3. PE transposes 12 [128,128] blocks: (t-pair, k) free -> partitions,
   giving Vk [(h,k)=128, (c, tau, 128 p-cols)] in PSUM; ACT copies to SBUF
4. DVE builds 34 fp16 arrays in SBUF: masks via is_gt (4x), products via
   TT mult (2x), relu lattice via tensor_scalar_max (4x), squares (2x)
5. PE: 35 accumulating matmuls with Moebius-weighted {0,+-1} stationaries:
   fuses the K-reduction AND the octant inclusion-exclusion butterfly into
   one PSUM accumulation chain MBS [76, 512] f32
6. ACT copy -> PE transposes back -> FIN [128 pair-cols, (tau,m',h)]
7. Small DVE/ACT epilogue (count clamp, reciprocal, means, std) + DMA out

Array index m = c*8 + T (T = mask-subset bitmask: 4=mx, 2=my, 1=mz).
Output rows m' = u*3+c (24 octant sums), 24+u (8 counts), 32+c (sum sq),
35+c (plain sums).
"""

import os
from contextlib import ExitStack

import numpy as np

if "axon" not in os.environ.get("JAX_PLATFORMS", "axon"):
    os.environ.pop("JAX_PLATFORMS", None)

import concourse.bass as bass
import concourse.bacc as bacc
import concourse.tile as tile
from concourse import mybir
from concourse.bass_utils import run_bass_kernel_spmd

B, C, N, K = 32, 3, 8192, 64
NCORES = 8
BL = B // NCORES          # 4 batches per core
PART = 128
SLAB = 1024               # groups per stripe
NSTRIPE = BL * N // SLAB  # 32
FOUT = 30
J = 512                   # pair-columns per stripe
NARR = 35                 # moving arrays per stripe
NOUT = 38                 # output feature rows per group-half
QCOL = NOUT * 2           # 76 stationary columns

AL = mybir.AluOpType
AF = mybir.ActivationFunctionType
F32 = mybir.dt.float32
FP16 = mybir.dt.float16


def _moebius_weights() -> np.ndarray:
    """W[a, m'] over 35 arrays x 38 outputs."""
    W = np.zeros((NARR, NOUT), dtype=np.float32)

    def pc(x):
        return bin(x).count("1")

    def moeb(u, T):
        # octant sum: oct[u] = sum_{T >= u} (-1)^{|T|-|u|} S_T
        if (T & u) == u:
            return float((-1) ** (pc(T) - pc(u)))
        return 0.0

    # value arrays a = c*8 + T
    for c in range(3):
        for T in range(8):
            a = c * 8 + T
            for u in range(8):
                W[a, u * 3 + c] = moeb(u, T)
            if T == 0:
                W[a, 35 + c] = 1.0  # plain sum passthrough
    # count arrays: a = 24..30 for T in (1,2,4,3,5,6,7); a=34 ones (T=0)
    cnt_T = [1, 2, 4, 3, 5, 6, 7]
    for i, T in enumerate(cnt_T):
        for u in range(8):
            W[24 + i, 24 + u] = moeb(u, T)
    for u in range(8):
        W[34, 24 + u] = moeb(u, 0)
    # squares a = 31+c
    for c in range(3):
        W[31 + c, 32 + c] = 1.0
    return W


def _stationaries() -> np.ndarray:
    """ST[p, a*76 + m'*2 + h]: Moebius weight, gated on h-block of p."""
    W = _moebius_weights()
    ST = np.zeros((PART, NARR * QCOL), dtype=np.float16)
    for a in range(NARR):
        for mp in range(NOUT):
            w = W[a, mp]
            if w == 0.0:
                continue
            for h in range(2):
                ST[h * 64:(h + 1) * 64, a * QCOL + mp * 2 + h] = w
    return ST


def _build_kernel(nc: bass.Bass):
    gx = nc.dram_tensor("gx", [BL, C, N, K], F32, kind="ExternalInput")
    nx = nc.dram_tensor("nx", [BL, N, C], F32, kind="ExternalInput")
    mst = nc.dram_tensor("mst", [PART, NARR * QCOL], FP16,
                         kind="ExternalInput")
    ident = nc.dram_tensor("ident", [PART, PART], FP16, kind="ExternalInput")
    out = nc.dram_tensor("out", [BL, N, FOUT], F32, kind="ExternalOutput")

    ts = None

    with tile.TileContext(nc) as tc, ExitStack() as ctx:
        spool = ctx.enter_context(tc.tile_pool(name="s", bufs=1))
        vpool = ctx.enter_context(tc.tile_pool(name="v", bufs=2))
        epool = ctx.enter_context(tc.tile_pool(name="e", bufs=2))
        pvk = ctx.enter_context(tc.tile_pool(name="pvk", bufs=1,
                                             space="PSUM"))
        pmb = ctx.enter_context(tc.tile_pool(name="pmb", bufs=2,
                                             space="PSUM"))
        pfn = ctx.enter_context(tc.tile_pool(name="pfn", bufs=2,
                                             space="PSUM"))

        ts = nc.vector.tensor_scalar
        tt = nc.vector.tensor_tensor
        act = nc.scalar.activation

        # static tiles
        ST = spool.tile([PART, NARR * QCOL], FP16, name="ST")
        ID = spool.tile([PART, PART], FP16, name="ID")
        IDF = spool.tile([PART, PART], F32, name="IDF")
        ONES = spool.tile([PART, J], FP16, name="ONES")
        nc.sync.dma_start(out=ST[:], in_=mst[:, :])
        nc.sync.dma_start(out=ID[:], in_=ident[:, :])
        act0 = nc.scalar.activation
        act0(IDF[:], ID[:], AF.Copy)
        nc.gpsimd.memset(ONES[:], 1.0)

        for s in range(NSTRIPE):
            b, blk = divmod(s, N // SLAB)
            n0 = blk * SLAB

            V = vpool.tile([PART, C * 512], F32, name="V")
            nc.sync.dma_start(
                out=V[:].rearrange("p (c t k) -> p c t k", c=C, t=8),
                in_=gx[b, :, n0:n0 + SLAB, :].rearrange(
                    "c (p t) k -> p c t k", p=PART, t=8))

            VH = vpool.tile([PART, C * 512], FP16, name="VH")
            act(VH[:], V[:], AF.Copy)

            # ---- transpose to [ (h,k), (c,tau,p) ] ----
            VKP = pvk.tile([PART, C * 512], FP16, name="VKP")
            for c in range(C):
                for tau in range(4):
                    o = c * 512 + tau * 128
                    nc.tensor.transpose(
                        VKP[:, o:o + 128], VH[:, o:o + 128], ID[:, :])
            VK = vpool.tile([PART, C * 512], FP16, name="VK")
            act(VK[:], VKP[:], AF.Copy)

            # ---- build arrays ----
            MK = vpool.tile([PART, C * J], FP16, name="MK")   # mx,my,mz
            MEGA = vpool.tile([PART, 24 * J], FP16, name="MEGA")
            CNT = vpool.tile([PART, 4 * J], FP16, name="CNT")
            SQ = vpool.tile([PART, C * J], FP16, name="SQ")

            vkc = VK[:].rearrange("p (c j) -> p c j", c=C)
            mkc = MK[:].rearrange("p (c j) -> p c j", c=C)
            mg = MEGA[:].rearrange("p (c t j) -> p c t j", c=C, t=8)

            # masks mx,my,mz (4x)
            ts(MK[:], VK[:], 0.0, None, AL.is_gt)

            # copy values into T=0 slots (4x)
            nc.vector.tensor_copy(mg[:, :, 0, :], vkc)

            def bcast(ap, n):
                return ap[:, None, :].broadcast_to([PART, n, J])

            # products (2x): dst m=c*8+T
            # P1: m1 (x*mz), m9 (y*mz)
            tt(mg[:, 0:2, 1, :], vkc[:, 0:2, :], bcast(mkc[:, 2, :], 2),
               AL.mult)
            # P2: m2 (x*my), m18 (z*my)
            mg24 = MEGA[:].rearrange("p (a j) -> p a j", a=24)
            tt(mg24[:, 2:24:16, :], vkc[:, 0:3:2, :],
               bcast(mkc[:, 1, :], 2), AL.mult)
            # P3: m3 = my * m1
            tt(mg[:, 0, 3, :], mg[:, 0, 1, :], mkc[:, 1, :], AL.mult)
            # P4: m12 (y*mx), m20 (z*mx)
            tt(mg[:, 1:3, 4, :], vkc[:, 1:3, :], bcast(mkc[:, 0, :], 2),
               AL.mult)
            # P5: m13 = mx * m9
            tt(mg[:, 1, 5, :], mg[:, 1, 1, :], mkc[:, 0, :], AL.mult)
            # P6: m22 = mx * m18
            tt(mg[:, 2, 6, :], mg[:, 2, 2, :], mkc[:, 0, :], AL.mult)

            # relus (4x): add own-coordinate mask
            nc.vector.tensor_scalar_max(mg[:, 0, 4:8, :], mg[:, 0, 0:4, :],
                                        0.0)
            my4 = MEGA[:].rearrange("p (c e d j) -> p c e d j",
                                    c=3, e=2, d=2)
            nc.vector.tensor_scalar_max(my4[:, 1, :, 1, :], my4[:, 1, :, 0, :],
                                        0.0)
            mz2 = MEGA[:].rearrange("p (c t d j) -> p c t d j",
                                    c=3, t=4, d=2)
            nc.vector.tensor_scalar_max(mz2[:, 2, :, 1, :], mz2[:, 2, :, 0, :],
                                        0.0)

            # count composite masks (4x): T=3 from m3? no:
            # CNT order T = 3,5,6,7 from is_gt of m9(z... see below
            # myz = 1[y*mz>0] (m9); mxz = 1[x*mz>0] (m1);
            # mxy = 1[x*my>0] (m2); mxyz = 1[x*my*mz>0] (m3)
            cn = CNT[:].rearrange("p (a j) -> p a j", a=4)
            ts(cn[:, 0, :], mg[:, 1, 1, :], 0.0, None, AL.is_gt)   # myz
            ts(cn[:, 1:4, :], mg[:, 0, 1:4, :], 0.0, None, AL.is_gt)

            # squares (2x)
            tt(SQ[:], VK[:], VK[:], AL.mult)

            # ---- PE reduce + Moebius ----
            MBS = pmb.tile([PART, J], F32, name="MBS")
            movs = ([mg[:, c, T, :] for c in range(3) for T in range(8)]
                    + [mkc[:, 2, :], mkc[:, 1, :], mkc[:, 0, :]]
                    + [cn[:, 0, :], cn[:, 1, :], cn[:, 2, :], cn[:, 3, :]]
                    + [SQ[:].rearrange("p (c j) -> p c j", c=3)[:, c, :]
                       for c in range(3)]
                    + [ONES[:]])
            # reorder: value arrays are mg[c][T] at a=c*8+T -> matches list
            for a, mov in enumerate(movs):
                nc.tensor.matmul(MBS[0:QCOL, :],
                                 ST[:, a * QCOL:(a + 1) * QCOL], mov,
                                 start=(a == 0), stop=(a == NARR - 1))

            # ---- transpose back ----
            MBSS = epool.tile([PART, J], F32, name="MBSS")
            act(MBSS[0:QCOL, :], MBS[0:QCOL, :], AF.Copy)
            FIN = pfn.tile([PART, 4 * QCOL], F32, name="FIN")
            for tau in range(4):
                nc.tensor.transpose(
                    FIN[:, tau * QCOL:(tau + 1) * QCOL],
                    MBSS[0:QCOL, tau * 128:(tau + 1) * 128],
                    IDF[0:QCOL, 0:QCOL])

            # ---- epilogue ----
            fin = FIN[:].rearrange("p (t m h) -> p t m h", t=4, m=NOUT)
            O = epool.tile([PART, 4 * 2 * FOUT], F32, name="O")
            ov = O[:].rearrange("p (t h f) -> p t h f", t=4, h=2)

            nc.sync.dma_start(
                out=ov[:, :, :, 3:6],
                in_=nx[b, n0:n0 + SLAB, :].rearrange(
                    "(p t h) c -> p t h c", p=PART, t=4))

            CC = epool.tile([PART, 64], F32, name="CC")
            RC = epool.tile([PART, 64], F32, name="RC")
            nc.vector.tensor_scalar_max(
                CC[:].rearrange("p (t u h) -> p t u h", t=4, u=8),
                fin[:, :, 24:32, :], 1.0)
            nc.vector.reciprocal_approx_fast(RC[:], CC[:])

            mnv = ov[:, :, :, 6:30].rearrange(
                "p t h (u c) -> p t u c h", u=8)
            sumv = fin[:, :, 0:24, :].rearrange(
                "p t (u c) h -> p t u c h", u=8)
            rcv = RC[:].rearrange("p (t u h) -> p t u h", t=4, u=8)
            rcb = rcv[:, :, :, None, :].broadcast_to([PART, 4, 8, 3, 2])
            tt(mnv, sumv, rcb, AL.mult)

            Q = epool.tile([PART, 24], F32, name="Q")
            qv = Q[:].rearrange("p (t c h) -> p t c h", t=4, c=3)
            act(qv, fin[:, :, 35:38, :], AF.Square, 0.0, 1.0 / 8.0)
            D = epool.tile([PART, 24], F32, name="D")
            dv = D[:].rearrange("p (t c h) -> p t c h", t=4, c=3)
            tt(dv, fin[:, :, 32:35, :], qv, AL.subtract)
            act(ov[:, :, :, 0:3].rearrange("p t h c -> p t c h"), dv,
                AF.Sqrt, 0.0, 1.0 / 63.0)

            nc.sync.dma_start(
                out=out[b, n0:n0 + SLAB, :].rearrange(
                    "(p t h) f -> p t h f", p=PART, t=4),
                in_=ov)


_CACHE: dict = {}


def _get_nc():
    if "nc" not in _CACHE:
        nc = bacc.Bacc("TRN2", target_bir_lowering=False, debug=False)
        _build_kernel(nc)
        nc.finalize()
        _CACHE["nc"] = nc
    return _CACHE["nc"]


def _consts():
    if "st" not in _CACHE:
        _CACHE["st"] = _stationaries()
        _CACHE["id"] = np.eye(PART, dtype=np.float16)
    return _CACHE["st"], _CACHE["id"]


def kernel(group_xyz: np.ndarray, new_xyz: np.ndarray) -> np.ndarray:
    nc = _get_nc()
    gx = np.ascontiguousarray(group_xyz, dtype=np.float32)
    nx = np.ascontiguousarray(new_xyz, dtype=np.float32)
    st, idm = _consts()
    in_maps = [
        {"gx": gx[i * BL:(i + 1) * BL], "nx": nx[i * BL:(i + 1) * BL],
         "mst": st, "ident": idm}
        for i in range(NCORES)
    ]
    res = run_bass_kernel_spmd(nc, in_maps, list(range(NCORES)))
    return np.concatenate([res.results[i]["out"] for i in range(NCORES)],
                          axis=0)
